# revision 19
# baseline (speedup 1.0000x reference)
"""GridRNN Trainium2 kernel.

Problem: 2-D grid RNN, B=4, S=T=128, H=256, D=3 depths.
  hx[d][b,i,j] = tanh(xin @ Wx_ih[d].T + bx_ih[d] + hx[d][b,i-1,(j-1)%T] @ Wx_hh[d].T + bx_hh[d])
  hy[d][b,i,j] = tanh(yin @ Wy_ih[d].T + by_ih[d] + hy[d][b,i,j-1]     @ Wy_hh[d].T + by_hh[d])
  (xin/yin = src/trg broadcast at d=0, previous depth's hx/hy for d>0)
  out = stack([hx[D-1], hy[D-1]], axis=-2)   # [B,S,T,2,H]

Key structure: the x-chain and y-chain never mix across depths -> 8 cores =
4 batches x 2 chains.  The x-chain's diagonal dependence hx[i-1,(j-1)%T] is
removed by shearing: u_i[c] = hx[i,(i+c)%T] turns it into a plain carry
u_{i-1}[c], identical in form to the y-chain.  One SPMD program runs on all
8 cores; only the input data (seed, weights) differs per core.  The host
unshears the x outputs and transposes the y outputs.

Per-step layout: state u as [128(part)=H%128, 2(k), V=128] in BF16 (PE runs
bf16 at 1 cycle/row vs fp32's 4; PSUM accumulates fp32; tolerance 2e-2).
Wavefront with depth offsets (0,2,4): tick t runs d0 step t, d1 step t-2,
d2 step t-4.  The 2-tick cross-depth slack lets each depth's input-term
matmuls run before the previous tick's activations complete.

v2 changes vs baseline:
 - All PE bias-opener matmuls (768 cyc/tick, 23% of PE) replaced by PSUM
   prefills on otherwise-idle engines: GPSIMD broadcasts the depth-0 input
   term pre0[:, :, t] into ps0, DVE broadcasts the d1/d2 bias columns into
   ps12.  PE matmuls then accumulate with start=False onto the prefill.
 - ACT split is configurable: "s3" = one tanh per depth (shortest
   recurrence loop: hh-mm + one [P,256] tanh + 2 sems per depth), "d12" =
   d0 + merged d1/d2 (fewer ACT fixed overheads, longer d1/d2 loop).
 - u1/u2 live in a tick-indexed SBUF ring (each slot written once, no pool
   rotation): ring[:, t, 0:2, :] = u2 step t-4, ring[:, t, 2:4, :] = u1
   step t-2.  Output DMA reads u2 slots strided straight from the ring.

Instruction ordering keeps every instruction at <= ONE sync-wait (walrus
limit): per tick PE issues [d1 ih (carries DVE-prefill wait), d2 ih
(covered), d2 hh (carries the tick's max ACT wait), d0 hh (carries Pool
wait; its ACT dep is covered by d2 hh), d1 hh (covered)].
"""

import numpy as np
import ml_dtypes

import concourse.bass as bass
import concourse.tile as tile
from concourse import mybir
from concourse.bass_utils import run_bass_kernel_spmd

B, S, T, H, D = 4, 128, 128, 256, 3
P = 128          # partitions
K = H // P       # 2 k-tiles of H on partitions
F32 = mybir.dt.float32
BF16 = mybir.dt.bfloat16
NPBF16 = np.dtype(ml_dtypes.bfloat16)
TANH = mybir.ActivationFunctionType.Tanh

ACT_SPLIT = "d12"         # "s3" (tanh per depth) or "d12" (d0 + merged d1d2)

# wblob (bf16) column layout
P0 = 0                    # pre0b: col 2*s+m at partition p = pre0[s, m*128+p]
W0 = P0 + 2 * S           # d0 whhT tiles: (k, m) -> k*H + m*P
WB = W0 + K * H           # d1/d2 wihT/whhT: (d-1, 0/1, k, m)
WCW = WB + 2 * 2 * K * H

# cblob (fp32): cols 0..3 = ps12 bias prefill in block order
# [d2m0, d2m1, d1m0, d1m1]; col 4 = zero (AP bias for activations)
ZCOL = 4
CCW = ZCOL + 1

OCHUNK = 16
NT = S + 4                # ticks 0..131

_cache = {}


def _patched_drain_and_barrier(self, tick_clock, wait_clock):
    """Replacement for TileContext._drain_and_barrier.

    This walrus build lowers at most ONE sync-wait per instruction; the stock
    tail drain carries one wait per active proc.  Semantically the waits only
    need to complete before the final barrier's semaphore cleanup, so spread
    them over single-wait NOPs on the sync engine after the drain.
    """
    drain_inst = self.nc.sync.drain()
    wait_clock.add_sem_waits(
        drain_inst.ins, tile.ScopedClock({None: tick_clock.global_clock})
    )
    ins = drain_inst.ins
    si = ins.sync_info
    if si is not None and len(si.on_wait) > 1:
        waits = list(si.on_wait)
        ins.sync_info = mybir.SyncInfo(on_wait=[waits[0]],
                                       on_update=list(si.on_update))
        for w in waits[1:]:
            nop = self.nc.sync.nop(nofuse=True)
            nop.ins.sync_info = mybir.SyncInfo(on_wait=[w], on_update=[])

    self.nc.all_engine_barrier()
    assert self.sems is not None
    popped = self.nc._tile_sem_poison_stack.pop()
    assert popped is self._sem_poison
    self.nc.clear_and_free_semaphores(list(self.sems.allocated().values()))
    self.nc.all_engine_barrier()


tile.TileContext._drain_and_barrier = _patched_drain_and_barrier


def _build():
    nc = bass.Bass(trn_type="TRN2")

    wblob = nc.dram_tensor("wblob", [P, WCW], BF16, kind="ExternalInput")
    cblob = nc.dram_tensor("cblob", [P, CCW], F32, kind="ExternalInput")
    # DRAM layout mirrors the ring's u2 cols ([p, s, k*T+v]) so the output
    # DMA is strided-contiguous 512B runs; host reassembles H = k*128+p.
    out = nc.dram_tensor("out", [P, S, K * T], BF16, kind="ExternalOutput")
    out_c = out[:, :, :]

    with tile.TileContext(nc) as tc:
        with (
            tc.tile_pool(name="consts", bufs=1) as consts,
            tc.tile_pool(name="u0p", bufs=4) as u0p,
            tc.tile_pool(name="psp", bufs=3, space="PSUM") as psp,
            tc.tile_pool(name="psi", bufs=1, space="PSUM") as psip,
        ):
            wb = consts.tile([P, WCW], BF16)
            cb = consts.tile([P, CCW], F32)
            nc.gpsimd.dma_start(out=wb[:, 0:WB], in_=wblob[:, 0:WB])
            nc.gpsimd.dma_start(out=cb, in_=cblob[:, :])
            nc.gpsimd.dma_start(out=wb[:, WB:], in_=wblob[:, WB:])
            # Pool absorbers: fold each input-DMA queue semaphore into Pool's
            # clock so later Pool-issued instructions carry no DMA waits.
            pscr = consts.tile([P, 2], BF16)
            pscr2 = consts.tile([P, 2], F32)
            nc.gpsimd.tensor_copy(out=pscr[:, 0:1], in_=wb[:, 0:1])
            nc.gpsimd.tensor_copy(out=pscr2[:, 0:1], in_=cb[:, 0:1])
            nc.gpsimd.tensor_copy(out=pscr[:, 1:2], in_=wb[:, WB:WB + 1])

            def wih(d, k, m):
                c = WB + ((d - 1) * 2) * K * H + k * H + m * P
                return wb[:, c:c + P]

            def whh(d, k, m):
                c = (W0 if d == 0 else WB + ((d - 1) * 2 + 1) * K * H) \
                    + k * H + m * P
                return wb[:, c:c + P]

            bias0 = cb[:, ZCOL:ZCOL + 1]

            # zeros on DVE so tick-0's d0 hh needs only a DVE wait (the DVE
            # clock is monotone: ps0-prefill(0) >= memset covers it)
            zeros = consts.tile([P, K, T], BF16)
            nc.vector.memset(zeros, 0.0)

            # DVE absorbers: fold the cblob and front-wblob DMA semaphores
            # into DVE's clock (DVE reads cb bias cols and wb's pre0b)
            vscr = consts.tile([P, 2], F32)
            vscrb = consts.tile([P, 2], BF16)
            nc.vector.tensor_copy(out=vscr[:, 0:1], in_=cb[:, 0:1])
            nc.vector.tensor_copy(out=vscrb[:, 0:1], in_=wb[:, 0:1])
            # ScalarE absorber (ACT reads cb's zero bias column)
            scr = consts.tile([P, 4], F32)
            nc.scalar.copy(out=scr[:, 0:1], in_=cb[:, 0:1])
            # PE absorbers: fold the two wblob DMAs into PE's clock
            dummy = psip.tile([32, 32], F32, tag="init")
            nc.tensor.matmul(dummy[:, :], lhsT=wb[0:32, 0:32], rhs=wb[0:32, 0:32],
                             start=True, stop=True)
            nc.tensor.matmul(dummy[:, :], lhsT=wb[0:32, WB:WB + 32],
                             rhs=wb[0:32, WB:WB + 32], start=True, stop=True)

            # ---- state storage
            # ring[:, t, 0:2, :] = u2 of step t-4 ; ring[:, t, 2:4, :] = u1
            # of step t-2 (both written at tick t; each slot written once).
            ring = consts.tile([P, NT, 4, T], BF16)

            def u1r(s):   # u1[s] view [P, 2(k), T]  (written at tick s+2)
                return ring[:, s + 2, 2:4, :]

            def u2r(s):   # u2[s] view [P, 2(k), T]  (written at tick s+4)
                return ring[:, s + 4, 0:2, :]

            u0 = {}
            u0[-1] = zeros

            def mm(ps_range, w, rhs, last):
                nc.tensor.matmul(ps_range, lhsT=w, rhs=rhs,
                                 start=False, stop=last,
                                 skip_group_check=True)

            # main wavefront, ticks 0..NT-1:
            #   d0 step t (t<=127), d1 step t-2 (2<=t<=129), d2 step t-4 (4<=t)
            for t in range(NT):
                s0_, s1_, s2_ = t, t - 2, t - 4
                do0 = s0_ <= S - 1
                do1 = 0 <= s1_ <= S - 1
                do2 = 0 <= s2_ <= S - 1

                # One unified PSUM tile per tick spanning 2 banks:
                # cols [d2m0 d2m1 | d1m0 d1m1 | d0m0 d0m1] -- no matmul
                # output straddles the bank boundary at col 512.
                ps = psp.tile([P, 6, T], F32, tag="ps")

                # -- DVE prefills.  pre0 (d0 input term) FIRST, bias cols
                # second: the first PE toucher of ps is an ih matmul whose
                # single DVE wait (>= the bias copy) then covers both.
                tc_ = min(t, S - 1)
                nc.vector.tensor_copy(
                    out=ps[:, 4:6, :],
                    in_=wb[:, P0 + 2 * tc_:P0 + 2 * tc_ + 2]
                        .broadcast_to([P, K, T]))
                nc.vector.tensor_copy(
                    out=ps[:, 0:4, :], in_=cb[:, 0:4].broadcast_to([P, 4, T]))

                # -- PE: d1 input-term matmuls (rhs = u0[s1], 2 ticks old).
                # First matmul carries the DVE-prefill wait.
                if do1:
                    for m_ in range(K):
                        for k_ in range(K):
                            mm(ps[:, 2 + m_, :],
                               wih(1, k_, m_), u0[s1_][:, k_, :], False)

                # -- PE: d2 input-term matmuls (rhs = u1[s2], 2 ticks old)
                if do2:
                    for m_ in range(K):
                        for k_ in range(K):
                            mm(ps[:, m_, :],
                               wih(2, k_, m_), u1r(s2_)[:, k_, :], False)

                # -- PE: d2 recurrent matmuls (oldest ACT target of the tick)
                if do2:
                    rhs2 = (lambda k_: zeros[:, k_, :]) if s2_ == 0 else \
                        (lambda k_: u2r(s2_ - 1)[:, k_, :])
                    for m_ in range(K):
                        for k_ in range(K):
                            mm(ps[:, m_, :],
                               whh(2, k_, m_), rhs2(k_), k_ == K - 1)

                # -- PE: d1 recurrent matmuls
                if do1:
                    rhs1 = (lambda k_: zeros[:, k_, :]) if s1_ == 0 else \
                        (lambda k_: u1r(s1_ - 1)[:, k_, :])
                    for m_ in range(K):
                        for k_ in range(K):
                            mm(ps[:, 2 + m_, :],
                               whh(1, k_, m_), rhs1(k_), k_ == K - 1)

                # -- PE: tiny wait-carrier matmul reading the newest u-tile
                # of tick t-1.  It dedicates a single-wait instruction to the
                # tick's newest ACT edge (so no real matmul needs two waits)
                # and MUST sit late in the tick's PE stream: the in-order PE
                # queue would otherwise gate the whole tick behind d0-tanh.
                if t >= 1:
                    if t - 1 <= S - 1:
                        sl = u0[t - 1][0:32, 0, 0:32]
                    elif t - 3 <= S - 1:
                        sl = ring[0:32, t - 1, 2, 0:32]
                    else:
                        sl = ring[0:32, t - 1, 0, 0:32]
                    nc.tensor.matmul(dummy[:, :], lhsT=sl, rhs=sl,
                                     start=True, stop=True)

                # -- PE: d0 recurrent matmuls (newest ACT target -> last)
                if do0:
                    for m_ in range(K):
                        for k_ in range(K):
                            mm(ps[:, 4 + m_, :], whh(0, k_, m_),
                               u0[s0_ - 1][:, k_, :], k_ == K - 1)

                # ACT order d2, d1, d0: each depth's hh matmuls of tick t+1
                # then have ascending ACT targets (d2-hh oldest, d0-hh
                # newest), so the list scheduler naturally orders them and
                # every instruction needs at most one fresh wait.
                if ACT_SPLIT == "s3":
                    # -- ACT: d2 tanh -> ring u2 slot, then d1 tanh -> u1 slot
                    if do2:
                        nc.scalar.activation(ring[:, t, 0:2, :],
                                             ps[:, 0:2, :], TANH, bias=bias0)
                    if do1:
                        nc.scalar.activation(ring[:, t, 2:4, :],
                                             ps[:, 2:4, :], TANH, bias=bias0)
                else:
                    # -- ACT: merged d1+d2 tanh -> full ring slot
                    if do1 or do2:
                        nc.scalar.activation(ring[:, t, :, :], ps[:, 0:4, :],
                                             TANH, bias=bias0)

                # -- ACT: d0 tanh -> u0 tile (bf16), last (tightest loop)
                if do0:
                    u = u0p.tile([P, K, T], BF16, tag="u0")
                    nc.scalar.activation(u, ps[:, 4:6, :], TANH, bias=bias0)
                    u0[s0_] = u

                # -- DMA out finished u2 chunks
                if do2 and (s2_ + 1) % OCHUNK == 0:
                    a = s2_ + 1 - OCHUNK
                    nc.gpsimd.dma_start(
                        out=out_c[:, a:a + OCHUNK, :],
                        in_=ring[:, a + 4:a + 4 + OCHUNK, 0:2, :])

                u0.pop(t - 4, None)

    _reduce_waits(nc)
    return nc


def _reduce_waits(nc):
    """Transitive reduction of semaphore waits (vector-clock based).

    This walrus build lowers at most ONE sync-wait per hardware instruction,
    but Tile emits a wait per dependency edge and does not elide waits that
    are transitively implied across engines (e.g. ACT waiting on both the
    PE group-close and the DVE PSUM-prefill the PE matmuls already waited
    on).  Compute each instruction's happens-before vector clock over the
    per-engine instruction sequences and drop any wait whose target event
    is already covered by the same-engine predecessor plus the remaining
    waits.  DMA-queue semaphores fire at transfer completion (async w.r.t.
    the issuing engine), so waits on them are used for coverage credit but
    never themselves dropped.
    """
    instrs = [ins for blk in nc.m.functions[0].blocks
              for ins in blk.instructions]
    seq = {}
    pos = {}
    for ins in instrs:
        eng = str(ins.engine)
        pos[id(ins)] = (eng, len(seq.setdefault(eng, [])))
        seq[eng].append(ins)

    # sem name -> updater proc + list of (cum_value, instr_idx)
    sem_updaters = {}
    sem_cum = {}
    unsafe_sems = set()
    for eng, lst in seq.items():
        for i, ins in enumerate(lst):
            si = ins.sync_info
            if si is None:
                continue
            for u in si.on_update:
                name = str(u.ant_name)
                if u.sync_type != "semaphore" or u.update_mode != "sem-inc":
                    unsafe_sems.add(name)
                    continue
                if name in sem_updaters and sem_updaters[name] != eng:
                    unsafe_sems.add(name)
                    continue
                sem_updaters[name] = eng
                cum = sem_cum.get(name, 0) + int(u.update_value)
                sem_cum[name] = cum
                sem_updaters.setdefault((name, "ev"), []).append((cum, i))

    def event_of(name, value):
        """(proc, idx) of the instruction whose completion makes sem>=value,
        or None if unanalyzable."""
        if name in unsafe_sems:
            return None
        evs = sem_updaters.get((name, "ev"))
        if not evs:
            return None
        for cum, idx in evs:
            if cum >= value:
                return (sem_updaters[name], idx)
        return None

    dma_sem = {name for name in sem_updaters
               if isinstance(name, str) and name.startswith("DMASW")}

    # vector clocks: vc[(eng, idx)] = dict proc -> last completed idx
    vc = {}
    done = {}
    nexti = {eng: 0 for eng in seq}
    progress = True
    while progress:
        progress = False
        for eng in seq:
            while nexti[eng] < len(seq[eng]):
                i = nexti[eng]
                ins = seq[eng][i]
                base = dict(vc.get((eng, i - 1), {})) if i else {}
                if i:
                    base[eng] = i - 1
                ok = True
                si = ins.sync_info
                if si is not None:
                    for w in si.on_wait:
                        if w.sync_type != "semaphore" or \
                                w.wait_mode != "sem-ge-imm":
                            continue
                        ev = event_of(str(w.ant_name), int(w.wait_value))
                        if ev is None:
                            continue
                        peng, pidx = ev
                        if peng not in done or done[peng] < pidx:
                            ok = False
                            break
                        evvc = vc.get((peng, pidx), {})
                        for k, v in evvc.items():
                            if base.get(k, -1) < v:
                                base[k] = v
                        if base.get(peng, -1) < pidx:
                            base[peng] = pidx
                    if not ok:
                        break
                vc[(eng, i)] = base
                done[eng] = i
                nexti[eng] = i + 1
                progress = True

    # drop covered waits
    for eng in seq:
        for i, ins in enumerate(seq[eng]):
            si = ins.sync_info
            if si is None or len(si.on_wait) <= 1 or (eng, i) not in vc:
                continue
            # dedupe identical (sem, value) waits first -- two identical
            # waits would otherwise "cover" each other and both be dropped
            waits = []
            seen = set()
            for w in si.on_wait:
                key = (str(w.ant_name), getattr(w, "wait_value", None),
                       w.sync_type, getattr(w, "wait_mode", None))
                if key in seen:
                    continue
                seen.add(key)
                waits.append(w)
            keep = []
            for j, w in enumerate(waits):
                if w.sync_type != "semaphore" or w.wait_mode != "sem-ge-imm" \
                        or str(w.ant_name) in dma_sem:
                    keep.append(w)
                    continue
                ev = event_of(str(w.ant_name), int(w.wait_value))
                if ev is None:
                    keep.append(w)
                    continue
                # coverage: predecessor + all OTHER waits (kept or not-yet-
                # decided ones still provide their edges -- they are only
                # dropped when themselves covered, preserving the relation)
                cover = dict(vc.get((eng, i - 1), {})) if i else {}
                if i:
                    cover[eng] = i - 1
                for k2, w2 in enumerate(waits):
                    if k2 == j or w2.sync_type != "semaphore" \
                            or w2.wait_mode != "sem-ge-imm":
                        continue
                    ev2 = event_of(str(w2.ant_name), int(w2.wait_value))
                    if ev2 is None:
                        continue
                    p2, i2 = ev2
                    evvc = vc.get((p2, i2), {})
                    for k, v in evvc.items():
                        if cover.get(k, -1) < v:
                            cover[k] = v
                    if cover.get(p2, -1) < i2:
                        cover[p2] = i2
                peng, pidx = ev
                if cover.get(peng, -1) >= pidx:
                    continue  # transitively covered -> drop
                keep.append(w)
            if len(keep) != len(si.on_wait):
                ins.sync_info = mybir.SyncInfo(
                    on_wait=keep, on_update=list(si.on_update))


def _wblob(seed, wT_ih, wT_hh, bs):
    """Pack per-core bf16 constants into the [P, WCW] weights blob.

    pre0 (depth-0 input term + bias) is computed here on the host and
    stored transposed/interleaved for the per-tick GPSIMD prefill."""
    b = np.empty((P, WCW), NPBF16)

    def wtile(wT, d):  # wT[d] -> [P, K*H] with (k, m) -> k*H + m*P
        return wT[d].reshape(K, P, H).transpose(1, 0, 2).reshape(P, K * H)

    # pre0b[p, 2*s+m] = (seed @ W_ih[0].T + bsum[0])[s, m*128+p]
    pre0 = (seed @ wT_ih[0] + bs[0]).astype(np.float32)   # [S, H]
    b[:, P0:P0 + 2 * S] = pre0.reshape(S, K, P).transpose(2, 0, 1) \
        .reshape(P, S * K)
    b[:, W0:W0 + K * H] = wtile(wT_hh, 0)
    for d in range(1, D):
        base = WB + (d - 1) * 2 * K * H
        b[:, base:base + K * H] = wtile(wT_ih, d)
        b[:, base + K * H:base + 2 * K * H] = wtile(wT_hh, d)
    return b


def _cblob(bs):
    """fp32 bias columns in ps12 block order [d2m0, d2m1, d1m0, d1m1] + a
    zero column (AP bias for the activations)."""
    c = np.empty((P, CCW), np.float32)
    for col, (d, m) in enumerate([(2, 0), (2, 1), (1, 0), (1, 1)]):
        c[:, col] = bs[d][m * P:(m + 1) * P]
    c[:, ZCOL] = 0.0
    return c


def kernel(src, trg, Wx_ih, Wx_hh, bx_ih, bx_hh, Wy_ih, Wy_hh, by_ih, by_hh):
    if "nc" not in _cache:
        _cache["nc"] = _build()
    nc = _cache["nc"]

    def tr(w):  # [D,H,H] -> W[d].T contiguous
        return np.ascontiguousarray(np.swapaxes(np.asarray(w, np.float32), 1, 2))

    src = np.asarray(src, np.float32)
    trg = np.asarray(trg, np.float32)
    wx_ihT, wx_hhT = tr(Wx_ih), tr(Wx_hh)
    wy_ihT, wy_hhT = tr(Wy_ih), tr(Wy_hh)
    bx = np.asarray(bx_ih, np.float32) + np.asarray(bx_hh, np.float32)
    by = np.asarray(by_ih, np.float32) + np.asarray(by_hh, np.float32)

    in_maps = []
    for b in range(B):  # cores 0-3: x chains
        in_maps.append({"wblob": _wblob(src[b], wx_ihT, wx_hhT, bx),
                        "cblob": _cblob(bx)})
    for b in range(B):  # cores 4-7: y chains
        in_maps.append({"wblob": _wblob(trg[b], wy_ihT, wy_hhT, by),
                        "cblob": _cblob(by)})

    _cache["last_in_maps"] = in_maps
    globals()["_last_in_maps"] = in_maps
    # Warmup execution: the very first NEFF execution in a process can hit a
    # cold-start timing transient; discard it and return the steady run.
    run_bass_kernel_spmd(nc, in_maps, list(range(8)))
    res = run_bass_kernel_spmd(nc, in_maps, list(range(8)))

    out = np.empty((B, S, T, 2, H), np.float32)
    ii = np.arange(S)[:, None]
    jj = np.arange(T)[None, :]
    idx = (jj - ii) % T  # hx[i,j] = u_i[(j-i)%T]
    for b in range(B):
        # raw core output [p, s, k*T+v] -> [s, H=k*128+p, v]
        arr = np.asarray(res.results[b]["out"]).astype(np.float32)
        arr = arr.reshape(P, S, K, T).transpose(1, 2, 0, 3).reshape(S, H, T)
        hx = np.take_along_axis(arr, idx[:, None, :], axis=2)  # [s, H, j]
        out[b, :, :, 0, :] = hx.transpose(0, 2, 1)
        arr = np.asarray(res.results[B + b]["out"]).astype(np.float32)
        arr = arr.reshape(P, S, K, T).transpose(1, 2, 0, 3).reshape(S, H, T)
        out[b, :, :, 1, :] = arr.transpose(2, 0, 1)  # [j, H, i] -> [i, j, H]
    return out


# revision 27
# speedup vs baseline: 1.2684x; 1.2684x over previous
"""GridRNN Trainium2 kernel.

Problem: 2-D grid RNN, B=4, S=T=128, H=256, D=3 depths.
  hx[d][b,i,j] = tanh(xin @ Wx_ih[d].T + bx_ih[d] + hx[d][b,i-1,(j-1)%T] @ Wx_hh[d].T + bx_hh[d])
  hy[d][b,i,j] = tanh(yin @ Wy_ih[d].T + by_ih[d] + hy[d][b,i,j-1]     @ Wy_hh[d].T + by_hh[d])
  (xin/yin = src/trg broadcast at d=0, previous depth's hx/hy for d>0)
  out = stack([hx[D-1], hy[D-1]], axis=-2)   # [B,S,T,2,H]

Key structure: the x-chain and y-chain never mix across depths -> 8 cores =
4 batches x 2 chains.  The x-chain's diagonal dependence hx[i-1,(j-1)%T] is
removed by shearing: u_i[c] = hx[i,(i+c)%T] turns it into a plain carry
u_{i-1}[c], identical in form to the y-chain.  One SPMD program runs on all
8 cores; only the input data (seed, weights) differs per core.  The host
unshears the x outputs and transposes the y outputs.

Per-step layout: state u as [128(part)=H%128, 2(k), V=128] in BF16 (PE runs
bf16 at 1 cycle/row vs fp32's 4; PSUM accumulates fp32; tolerance 2e-2).
Wavefront with depth offsets (0,2,4): tick t runs d0 step t, d1 step t-2,
d2 step t-4.  The 2-tick cross-depth slack lets each depth's input-term
matmuls run before the previous tick's activations complete.

v2 changes vs baseline:
 - All PE bias-opener matmuls (768 cyc/tick, 23% of PE) replaced by PSUM
   prefills on otherwise-idle engines: GPSIMD broadcasts the depth-0 input
   term pre0[:, :, t] into ps0, DVE broadcasts the d1/d2 bias columns into
   ps12.  PE matmuls then accumulate with start=False onto the prefill.
 - ACT split is configurable: "s3" = one tanh per depth (shortest
   recurrence loop: hh-mm + one [P,256] tanh + 2 sems per depth), "d12" =
   d0 + merged d1/d2 (fewer ACT fixed overheads, longer d1/d2 loop).
 - u1/u2 live in a tick-indexed SBUF ring (each slot written once, no pool
   rotation): ring[:, t, 0:2, :] = u2 step t-4, ring[:, t, 2:4, :] = u1
   step t-2.  Output DMA reads u2 slots strided straight from the ring.

Instruction ordering keeps every instruction at <= ONE sync-wait (walrus
limit): per tick PE issues [d1 ih (carries DVE-prefill wait), d2 ih
(covered), d2 hh (carries the tick's max ACT wait), d0 hh (carries Pool
wait; its ACT dep is covered by d2 hh), d1 hh (covered)].
"""

import numpy as np
import ml_dtypes

import concourse.bass as bass
import concourse.tile as tile
from concourse import mybir
from concourse.bass_utils import run_bass_kernel_spmd

B, S, T, H, D = 4, 128, 128, 256, 3
P = 128          # partitions
K = H // P       # 2 k-tiles of H on partitions
F32 = mybir.dt.float32
BF16 = mybir.dt.bfloat16
NPBF16 = np.dtype(ml_dtypes.bfloat16)
TANH = mybir.ActivationFunctionType.Tanh

ACT_SPLIT = "d12"         # "s3" (tanh per depth) or "d12" (d0 + merged d1d2)

# wblob (bf16) column layout
P0 = 0                    # pre0b: col 2*s+m at partition p = pre0[s, m*128+p]
W0 = P0 + 2 * S           # d0 whhT tiles: (k, m) -> k*H + m*P
WB = W0 + K * H           # d1/d2 wihT/whhT: (d-1, 0/1, k, m)
WCW = WB + 2 * 2 * K * H

# cblob (fp32): cols 0..3 = ps12 bias prefill in block order
# [d2m0, d2m1, d1m0, d1m1]; col 4 = zero (AP bias for activations)
ZCOL = 4
CCW = ZCOL + 1

OCHUNK = 16
NT = S + 4                # ticks 0..131

_cache = {}


def _patched_drain_and_barrier(self, tick_clock, wait_clock):
    """Replacement for TileContext._drain_and_barrier.

    This walrus build lowers at most ONE sync-wait per instruction; the stock
    tail drain carries one wait per active proc.  Semantically the waits only
    need to complete before the final barrier's semaphore cleanup, so spread
    them over single-wait NOPs on the sync engine after the drain.
    """
    drain_inst = self.nc.sync.drain()
    wait_clock.add_sem_waits(
        drain_inst.ins, tile.ScopedClock({None: tick_clock.global_clock})
    )
    ins = drain_inst.ins
    si = ins.sync_info
    if si is not None and len(si.on_wait) > 1:
        waits = list(si.on_wait)
        ins.sync_info = mybir.SyncInfo(on_wait=[waits[0]],
                                       on_update=list(si.on_update))
        for w in waits[1:]:
            nop = self.nc.sync.nop(nofuse=True)
            nop.ins.sync_info = mybir.SyncInfo(on_wait=[w], on_update=[])

    self.nc.all_engine_barrier()
    assert self.sems is not None
    popped = self.nc._tile_sem_poison_stack.pop()
    assert popped is self._sem_poison
    self.nc.clear_and_free_semaphores(list(self.sems.allocated().values()))
    self.nc.all_engine_barrier()


tile.TileContext._drain_and_barrier = _patched_drain_and_barrier


def _build():
    nc = bass.Bass(trn_type="TRN2")

    wblob = nc.dram_tensor("wblob", [P, WCW], BF16, kind="ExternalInput")
    cblob = nc.dram_tensor("cblob", [P, CCW], F32, kind="ExternalInput")
    # DRAM layout mirrors the ring's u2 cols ([p, s, k*T+v]) so the output
    # DMA is strided-contiguous 512B runs; host reassembles H = k*128+p.
    out = nc.dram_tensor("out", [P, S, K * T], BF16, kind="ExternalOutput")
    out_c = out[:, :, :]

    with tile.TileContext(nc) as tc:
        with (
            tc.tile_pool(name="consts", bufs=1) as consts,
            tc.tile_pool(name="u0p", bufs=4) as u0p,
            tc.tile_pool(name="ps12", bufs=3, space="PSUM") as ps12p,
            tc.tile_pool(name="ps0", bufs=3, space="PSUM") as ps0p,
            tc.tile_pool(name="psi", bufs=1, space="PSUM") as psip,
        ):
            wb = consts.tile([P, WCW], BF16)
            cb = consts.tile([P, CCW], F32)
            nc.gpsimd.dma_start(out=wb[:, 0:WB], in_=wblob[:, 0:WB])
            nc.gpsimd.dma_start(out=cb, in_=cblob[:, :])
            nc.gpsimd.dma_start(out=wb[:, WB:], in_=wblob[:, WB:])
            # Pool absorbers: fold each input-DMA queue semaphore into Pool's
            # clock so later Pool-issued instructions carry no DMA waits.
            pscr = consts.tile([P, 2], BF16)
            pscr2 = consts.tile([P, 2], F32)
            nc.gpsimd.tensor_copy(out=pscr[:, 0:1], in_=wb[:, 0:1])
            nc.gpsimd.tensor_copy(out=pscr2[:, 0:1], in_=cb[:, 0:1])
            nc.gpsimd.tensor_copy(out=pscr[:, 1:2], in_=wb[:, WB:WB + 1])

            def wih(d, k, m):
                c = WB + ((d - 1) * 2) * K * H + k * H + m * P
                return wb[:, c:c + P]

            def whh(d, k, m):
                c = (W0 if d == 0 else WB + ((d - 1) * 2 + 1) * K * H) \
                    + k * H + m * P
                return wb[:, c:c + P]

            bias0 = cb[:, ZCOL:ZCOL + 1]

            # zeros on DVE so tick-0's d0 hh needs only a DVE wait (the DVE
            # clock is monotone: ps0-prefill(0) >= memset covers it)
            zeros = consts.tile([P, K, T], BF16)
            nc.vector.memset(zeros, 0.0)

            # DVE absorbers: fold the cblob and front-wblob DMA semaphores
            # into DVE's clock (DVE reads cb bias cols and wb's pre0b)
            vscr = consts.tile([P, 2], F32)
            vscrb = consts.tile([P, 2], BF16)
            nc.vector.tensor_copy(out=vscr[:, 0:1], in_=cb[:, 0:1])
            nc.vector.tensor_copy(out=vscrb[:, 0:1], in_=wb[:, 0:1])
            # ScalarE absorber (ACT reads cb's zero bias column)
            scr = consts.tile([P, 4], F32)
            nc.scalar.copy(out=scr[:, 0:1], in_=cb[:, 0:1])
            # PE absorbers: fold the two wblob DMAs into PE's clock
            dummy = psip.tile([32, 32], F32, tag="init")
            nc.tensor.matmul(dummy[:, :], lhsT=wb[0:32, 0:32], rhs=wb[0:32, 0:32],
                             start=True, stop=True)
            nc.tensor.matmul(dummy[:, :], lhsT=wb[0:32, WB:WB + 32],
                             rhs=wb[0:32, WB:WB + 32], start=True, stop=True)

            # ---- state storage
            # ring[:, t, 0:2, :] = u2 of step t-4 ; ring[:, t, 2:4, :] = u1
            # of step t-2 (both written at tick t; each slot written once).
            ring = consts.tile([P, NT, 4, T], BF16)

            def u1r(s):   # u1[s] view [P, 2(k), T]  (written at tick s+2)
                return ring[:, s + 2, 2:4, :]

            def u2r(s):   # u2[s] view [P, 2(k), T]  (written at tick s+4)
                return ring[:, s + 4, 0:2, :]

            u0 = {}
            u0[-1] = zeros

            def mm(ps_range, w, rhs, last):
                nc.tensor.matmul(ps_range, lhsT=w, rhs=rhs,
                                 start=False, stop=last,
                                 skip_group_check=True)

            # main wavefront, ticks 0..NT-1:
            #   d0 step t (t<=127), d1 step t-2 (2<=t<=129), d2 step t-4 (4<=t)
            for t in range(NT):
                s0_, s1_, s2_ = t, t - 2, t - 4
                do0 = s0_ <= S - 1
                do1 = 0 <= s1_ <= S - 1
                do2 = 0 <= s2_ <= S - 1

                # Two PSUM tiles per tick: ps12 = [d2m0 d2m1 | d1m0 d1m1]
                # (one bank), ps0 = [d0m0 d0m1] (half bank).  Separate tiles
                # keep the tanh->writer dependencies precise (the unified
                # 6-region tile defeated Tile's sub-tile tracking and glued
                # every tanh to every matmul).
                ps12 = ps12p.tile([P, 4, T], F32, tag="ps12")
                ps0 = ps0p.tile([P, K, T], F32, tag="ps0")

                # -- DVE prefills.  pre0 (d0 input term) FIRST, bias cols
                # second: the first PE toucher of the tick is an ih matmul
                # whose single DVE wait (>= the bias copy) then covers both.
                tc_ = min(t, S - 1)
                nc.vector.tensor_copy(
                    out=ps0,
                    in_=wb[:, P0 + 2 * tc_:P0 + 2 * tc_ + 2]
                        .broadcast_to([P, K, T]))
                nc.vector.tensor_copy(
                    out=ps12, in_=cb[:, 0:4].broadcast_to([P, 4, T]))

                # -- PE: d1 input-term matmuls (rhs = u0[s1], 2 ticks old).
                # First matmul carries the DVE-prefill wait.
                if do1:
                    for m_ in range(K):
                        for k_ in range(K):
                            mm(ps12[:, 2 + m_, :],
                               wih(1, k_, m_), u0[s1_][:, k_, :], False)

                # -- PE: d2 input-term matmuls (rhs = u1[s2], 2 ticks old)
                if do2:
                    for m_ in range(K):
                        for k_ in range(K):
                            mm(ps12[:, m_, :],
                               wih(2, k_, m_), u1r(s2_)[:, k_, :], False)

                # -- PE: d2 recurrent matmuls (oldest ACT target of the tick)
                if do2:
                    rhs2 = (lambda k_: zeros[:, k_, :]) if s2_ == 0 else \
                        (lambda k_: u2r(s2_ - 1)[:, k_, :])
                    for m_ in range(K):
                        for k_ in range(K):
                            mm(ps12[:, m_, :],
                               whh(2, k_, m_), rhs2(k_), k_ == K - 1)

                # -- PE: d1 recurrent matmuls
                if do1:
                    rhs1 = (lambda k_: zeros[:, k_, :]) if s1_ == 0 else \
                        (lambda k_: u1r(s1_ - 1)[:, k_, :])
                    for m_ in range(K):
                        for k_ in range(K):
                            mm(ps12[:, 2 + m_, :],
                               whh(1, k_, m_), rhs1(k_), k_ == K - 1)

                # -- PE: tiny wait-carrier matmul reading the newest u-tile
                # of tick t-1.  It dedicates a single-wait instruction to the
                # tick's newest ACT edge (so no real matmul needs two waits)
                # and MUST sit late in the tick's PE stream: the in-order PE
                # queue would otherwise gate the whole tick behind d0-tanh.
                if t >= 1:
                    if t - 1 <= S - 1:
                        sl = u0[t - 1][0:32, 0, 0:32]
                    elif t - 3 <= S - 1:
                        sl = ring[0:32, t - 1, 2, 0:32]
                    else:
                        sl = ring[0:32, t - 1, 0, 0:32]
                    nc.tensor.matmul(dummy[:, :], lhsT=sl, rhs=sl,
                                     start=True, stop=True)

                # -- PE: d0 recurrent matmuls (newest ACT target -> last)
                if do0:
                    for m_ in range(K):
                        for k_ in range(K):
                            mm(ps0[:, m_, :], whh(0, k_, m_),
                               u0[s0_ - 1][:, k_, :], k_ == K - 1)

                # ACT order d2, d1, d0: each depth's hh matmuls of tick t+1
                # then have ascending ACT targets (d2-hh oldest, d0-hh
                # newest), so the list scheduler naturally orders them and
                # every instruction needs at most one fresh wait.
                if ACT_SPLIT == "s3":
                    # -- ACT: d2 tanh -> ring u2 slot, then d1 tanh -> u1 slot
                    if do2:
                        nc.scalar.activation(ring[:, t, 0:2, :],
                                             ps12[:, 0:2, :], TANH, bias=bias0)
                    if do1:
                        nc.scalar.activation(ring[:, t, 2:4, :],
                                             ps12[:, 2:4, :], TANH, bias=bias0)
                else:
                    # -- ACT: merged d1+d2 tanh -> full ring slot
                    if do1 or do2:
                        nc.scalar.activation(ring[:, t, :, :], ps12,
                                             TANH, bias=bias0)

                # -- ACT: d0 tanh -> u0 tile (bf16), last (tightest loop)
                if do0:
                    u = u0p.tile([P, K, T], BF16, tag="u0")
                    nc.scalar.activation(u, ps0, TANH, bias=bias0)
                    u0[s0_] = u

                # -- DMA out finished u2 chunks
                if do2 and (s2_ + 1) % OCHUNK == 0:
                    a = s2_ + 1 - OCHUNK
                    nc.gpsimd.dma_start(
                        out=out_c[:, a:a + OCHUNK, :],
                        in_=ring[:, a + 4:a + 4 + OCHUNK, 0:2, :])

                u0.pop(t - 4, None)

    _reduce_waits(nc)
    return nc


def _reduce_waits(nc):
    """Transitive reduction of semaphore waits (vector-clock based).

    This walrus build lowers at most ONE sync-wait per hardware instruction,
    but Tile emits a wait per dependency edge and does not elide waits that
    are transitively implied across engines (e.g. ACT waiting on both the
    PE group-close and the DVE PSUM-prefill the PE matmuls already waited
    on).  Compute each instruction's happens-before vector clock over the
    per-engine instruction sequences and drop any wait whose target event
    is already covered by the same-engine predecessor plus the remaining
    waits.  DMA-queue semaphores fire at transfer completion (async w.r.t.
    the issuing engine), so waits on them are used for coverage credit but
    never themselves dropped.
    """
    instrs = [ins for blk in nc.m.functions[0].blocks
              for ins in blk.instructions]
    seq = {}
    pos = {}
    for ins in instrs:
        eng = str(ins.engine)
        pos[id(ins)] = (eng, len(seq.setdefault(eng, [])))
        seq[eng].append(ins)

    # sem name -> updater proc + list of (cum_value, instr_idx)
    sem_updaters = {}
    sem_cum = {}
    unsafe_sems = set()
    for eng, lst in seq.items():
        for i, ins in enumerate(lst):
            si = ins.sync_info
            if si is None:
                continue
            for u in si.on_update:
                name = str(u.ant_name)
                if u.sync_type != "semaphore" or u.update_mode != "sem-inc":
                    unsafe_sems.add(name)
                    continue
                if name in sem_updaters and sem_updaters[name] != eng:
                    unsafe_sems.add(name)
                    continue
                sem_updaters[name] = eng
                cum = sem_cum.get(name, 0) + int(u.update_value)
                sem_cum[name] = cum
                sem_updaters.setdefault((name, "ev"), []).append((cum, i))

    def event_of(name, value):
        """(proc, idx) of the instruction whose completion makes sem>=value,
        or None if unanalyzable."""
        if name in unsafe_sems:
            return None
        evs = sem_updaters.get((name, "ev"))
        if not evs:
            return None
        for cum, idx in evs:
            if cum >= value:
                return (sem_updaters[name], idx)
        return None

    dma_sem = {name for name in sem_updaters
               if isinstance(name, str) and name.startswith("DMASW")}

    # vector clocks: vc[(eng, idx)] = dict proc -> last completed idx
    vc = {}
    done = {}
    nexti = {eng: 0 for eng in seq}
    progress = True
    while progress:
        progress = False
        for eng in seq:
            while nexti[eng] < len(seq[eng]):
                i = nexti[eng]
                ins = seq[eng][i]
                base = dict(vc.get((eng, i - 1), {})) if i else {}
                if i:
                    base[eng] = i - 1
                ok = True
                si = ins.sync_info
                if si is not None:
                    for w in si.on_wait:
                        if w.sync_type != "semaphore" or \
                                w.wait_mode != "sem-ge-imm":
                            continue
                        ev = event_of(str(w.ant_name), int(w.wait_value))
                        if ev is None:
                            continue
                        peng, pidx = ev
                        if peng not in done or done[peng] < pidx:
                            ok = False
                            break
                        evvc = vc.get((peng, pidx), {})
                        for k, v in evvc.items():
                            if base.get(k, -1) < v:
                                base[k] = v
                        if base.get(peng, -1) < pidx:
                            base[peng] = pidx
                    if not ok:
                        break
                vc[(eng, i)] = base
                done[eng] = i
                nexti[eng] = i + 1
                progress = True

    # drop covered waits
    for eng in seq:
        for i, ins in enumerate(seq[eng]):
            si = ins.sync_info
            if si is None or len(si.on_wait) <= 1 or (eng, i) not in vc:
                continue
            # dedupe identical (sem, value) waits first -- two identical
            # waits would otherwise "cover" each other and both be dropped
            waits = []
            seen = set()
            for w in si.on_wait:
                key = (str(w.ant_name), getattr(w, "wait_value", None),
                       w.sync_type, getattr(w, "wait_mode", None))
                if key in seen:
                    continue
                seen.add(key)
                waits.append(w)
            keep = []
            for j, w in enumerate(waits):
                if w.sync_type != "semaphore" or w.wait_mode != "sem-ge-imm" \
                        or str(w.ant_name) in dma_sem:
                    keep.append(w)
                    continue
                ev = event_of(str(w.ant_name), int(w.wait_value))
                if ev is None:
                    keep.append(w)
                    continue
                # coverage: predecessor + all OTHER waits (kept or not-yet-
                # decided ones still provide their edges -- they are only
                # dropped when themselves covered, preserving the relation)
                cover = dict(vc.get((eng, i - 1), {})) if i else {}
                if i:
                    cover[eng] = i - 1
                for k2, w2 in enumerate(waits):
                    if k2 == j or w2.sync_type != "semaphore" \
                            or w2.wait_mode != "sem-ge-imm":
                        continue
                    ev2 = event_of(str(w2.ant_name), int(w2.wait_value))
                    if ev2 is None:
                        continue
                    p2, i2 = ev2
                    evvc = vc.get((p2, i2), {})
                    for k, v in evvc.items():
                        if cover.get(k, -1) < v:
                            cover[k] = v
                    if cover.get(p2, -1) < i2:
                        cover[p2] = i2
                peng, pidx = ev
                if cover.get(peng, -1) >= pidx:
                    continue  # transitively covered -> drop
                keep.append(w)
            if len(keep) != len(si.on_wait):
                ins.sync_info = mybir.SyncInfo(
                    on_wait=keep, on_update=list(si.on_update))


def _wblob(seed, wT_ih, wT_hh, bs):
    """Pack per-core bf16 constants into the [P, WCW] weights blob.

    pre0 (depth-0 input term + bias) is computed here on the host and
    stored transposed/interleaved for the per-tick GPSIMD prefill."""
    b = np.empty((P, WCW), NPBF16)

    def wtile(wT, d):  # wT[d] -> [P, K*H] with (k, m) -> k*H + m*P
        return wT[d].reshape(K, P, H).transpose(1, 0, 2).reshape(P, K * H)

    # pre0b[p, 2*s+m] = (seed @ W_ih[0].T + bsum[0])[s, m*128+p]
    pre0 = (seed @ wT_ih[0] + bs[0]).astype(np.float32)   # [S, H]
    b[:, P0:P0 + 2 * S] = pre0.reshape(S, K, P).transpose(2, 0, 1) \
        .reshape(P, S * K)
    b[:, W0:W0 + K * H] = wtile(wT_hh, 0)
    for d in range(1, D):
        base = WB + (d - 1) * 2 * K * H
        b[:, base:base + K * H] = wtile(wT_ih, d)
        b[:, base + K * H:base + 2 * K * H] = wtile(wT_hh, d)
    return b


def _cblob(bs):
    """fp32 bias columns in ps12 block order [d2m0, d2m1, d1m0, d1m1] + a
    zero column (AP bias for the activations)."""
    c = np.empty((P, CCW), np.float32)
    for col, (d, m) in enumerate([(2, 0), (2, 1), (1, 0), (1, 1)]):
        c[:, col] = bs[d][m * P:(m + 1) * P]
    c[:, ZCOL] = 0.0
    return c


def kernel(src, trg, Wx_ih, Wx_hh, bx_ih, bx_hh, Wy_ih, Wy_hh, by_ih, by_hh):
    if "nc" not in _cache:
        _cache["nc"] = _build()
    nc = _cache["nc"]

    def tr(w):  # [D,H,H] -> W[d].T contiguous
        return np.ascontiguousarray(np.swapaxes(np.asarray(w, np.float32), 1, 2))

    src = np.asarray(src, np.float32)
    trg = np.asarray(trg, np.float32)
    wx_ihT, wx_hhT = tr(Wx_ih), tr(Wx_hh)
    wy_ihT, wy_hhT = tr(Wy_ih), tr(Wy_hh)
    bx = np.asarray(bx_ih, np.float32) + np.asarray(bx_hh, np.float32)
    by = np.asarray(by_ih, np.float32) + np.asarray(by_hh, np.float32)

    in_maps = []
    for b in range(B):  # cores 0-3: x chains
        in_maps.append({"wblob": _wblob(src[b], wx_ihT, wx_hhT, bx),
                        "cblob": _cblob(bx)})
    for b in range(B):  # cores 4-7: y chains
        in_maps.append({"wblob": _wblob(trg[b], wy_ihT, wy_hhT, by),
                        "cblob": _cblob(by)})

    _cache["last_in_maps"] = in_maps
    globals()["_last_in_maps"] = in_maps
    # Warmup execution: the very first NEFF execution in a process can hit a
    # cold-start timing transient; discard it and return the steady run.
    run_bass_kernel_spmd(nc, in_maps, list(range(8)))
    res = run_bass_kernel_spmd(nc, in_maps, list(range(8)))

    out = np.empty((B, S, T, 2, H), np.float32)
    ii = np.arange(S)[:, None]
    jj = np.arange(T)[None, :]
    idx = (jj - ii) % T  # hx[i,j] = u_i[(j-i)%T]
    for b in range(B):
        # raw core output [p, s, k*T+v] -> [s, H=k*128+p, v]
        arr = np.asarray(res.results[b]["out"]).astype(np.float32)
        arr = arr.reshape(P, S, K, T).transpose(1, 2, 0, 3).reshape(S, H, T)
        hx = np.take_along_axis(arr, idx[:, None, :], axis=2)  # [s, H, j]
        out[b, :, :, 0, :] = hx.transpose(0, 2, 1)
        arr = np.asarray(res.results[B + b]["out"]).astype(np.float32)
        arr = arr.reshape(P, S, K, T).transpose(1, 2, 0, 3).reshape(S, H, T)
        out[b, :, :, 1, :] = arr.transpose(2, 0, 1)  # [j, H, i] -> [i, j, H]
    return out
